# revision 34
# baseline (speedup 1.0000x reference)
"""Multi-head attention kernel for Trainium2, sharded over 8 NeuronCores.

Problem: Q,K,V [4, 16, 2048, 128] fp32 -> softmax(Q K^T / sqrt(128)) V.

Sharding: the 4*16 = 64 (batch, head) pairs are split across 8 cores,
8 pairs per core (pure data parallelism, no collectives).

Per-core kernel (flash-attention style, S^T layout), v3:
  - Q, K are loaded fp32, cast to fp16 (Q on DVE in 2 chunks, K on
    GpSimd), and transposed into Qt/Kt [d=128, seq] layout by the DMA
    XBAR (dma_start_transpose) -- no PE or PSUM involvement.
  - V is cast into V_aug [k, 129] tiles on GpSimd whose last column is
    1.0 (so the PV matmul also produces the softmax row sums for free).
  - Per q-chunk (W=256) the 16 k-tiles form 4 uniform groups of 4;
    S^T[k, q] group tiles [128, 1024] land in PSUM, rotating through 3
    two-bank buffers so QK(g+3) only waits on exp(g) -- deeper
    decoupling than the old (4,8,4)/A,B split and shorter exp latency
    per group, which removes the ~1.2us PE stalls at group boundaries.
  - exp(S^T/sqrt(d)): most groups go to the ACT engine (fp32 PSUM ->
    fp16 P^T in SBUF); DVE_G groups instead run on the Vector engine
    via two custom DVE ops: base = 1 + y(1 + y(c2 + y c3)) with
    y = s*scale/16, then base^16 (rel err ~2e-4 rms).  This keeps the
    saturated ACT engine just under the PE's per-pair time.
  - O_unnorm[q, 0:128] and row sums [q, 128] accumulate in PSUM over
    all k tiles via matmul(lhsT=P^T slice, rhs=V_aug).
  - Final normalize: reciprocal of sums on DVE, the O*recip scaling on
    GpSimd (keeps the DVE queue free for exp).
  - PSUM budget (8 banks): st 3x2 + two O accumulators 2 = 8.

Scheduling: a global software pipeline over the group stream (PV
matmuls trail the S^T/exp stream by 3 groups), with prep for later
pairs (loads two pairs ahead; casts + DMA transposes ~1.5 pairs ahead)
emitted into the per-q-chunk gaps so the ACT/PE pipeline never drains
at pair boundaries.
"""

import os
import re
import sys

for _p in ("/opt/trn_rl_repo",):
    if _p not in sys.path and os.path.isdir(_p):
        sys.path.insert(0, _p)

import numpy as np

import concourse.bass as bass
import concourse.bacc as bacc
import concourse.tile as tile
from concourse import mybir
from concourse import dve_ops as DO
from concourse.dve_spec import Spec, Src0, C0, C1, C2, One, sq
from concourse.dve_table_gen import dve_ver_for
from concourse.bass_utils import run_bass_kernel_spmd

F32 = mybir.dt.float32
F16 = mybir.dt.float16

B, H, S, D = 4, 16, 2048, 128
N_CORES = 8
PAIRS = (B * H) // N_CORES  # (b,h) pairs per core
P = 128  # partition dim / head dim / seq tile
W = 256  # q-chunk width (one PSUM bank of O accum incl. sums col)
DA = D + 1  # V augmented with a ones column
SCALE = float(1.0 / np.sqrt(D))
# deg-3 base coefficients for the DVE exp (Taylor; composite ^16)
EXP_C2, EXP_C3 = 0.5, 1.0 / 6.0
# (qc, group) pairs whose exp runs on DVE instead of ACT.  Group 3 so
# the previous q-chunk's normalize (same DVE queue) is emitted before
# the exp chain -- the normalize gates the O PSUM bank rotation and
# must not sit behind a 2.4us exp chain.
DVE_G = {(1, 3), (2, 3), (3, 3), (5, 3), (6, 3), (7, 3)}

_nc_cache = {}
_exp_ops = None


def _make_op(name, spec):
    """Build a DveOp with the correct uops_sha (sha comes from the
    drift error message on first compile)."""
    ver = dve_ver_for("TRN2")
    op = DO.DveOp(name, spec, subdim=False, uops_sha={ver: "?"})
    try:
        op.compile(ver)
    except ValueError as e:
        sha = re.search(r"([0-9a-f]{16,})", str(e)).group(1)
        op = DO.DveOp(name, spec, subdim=False, uops_sha={ver: sha})
    op.compile(ver)
    return op


def register_exp_ops():
    """Register the two custom DVE ops for exp(x*scale) = base(y)^16.

    base: 1 + y*(1 + y*(C1 + y*C2)), y = Src0*C0  (7 ALU stages)
    pow:  Src0^16                                  (4 ALU stages)
    """
    global _exp_ops
    if _exp_ops is not None:
        return _exp_ops
    if "EXP16_BASE_ANT" in DO._SUB_OPCODE_FOR_NAME:
        ops = {o.name: o for o in DO.OPS}
        _exp_ops = (ops["EXP16_BASE_ANT"], ops["EXP16_POW_ANT"])
        return _exp_ops
    y = Src0 * C0
    t = ((y * C2) + C1) * y
    p = ((t + One) * y) + One
    base_spec = Spec(
        body=p,
        reference=lambda in0, in1, s0, s1, imm2: (
            lambda yy: 1.0 + yy * (1.0 + yy * (s1 + yy * imm2))
        )(np.asarray(in0, np.float32) * (s0 if np.isscalar(s0) else s0)),
    )
    pow_spec = Spec(
        body=sq(sq(sq(sq(Src0)))),
        reference=lambda in0, in1, s0, s1, imm2: np.asarray(in0, np.float32) ** 16,
    )
    next_row = DO._CUSTOM_DVE_ROW_BASE + len(DO.OPS)
    DO._SUB_OPCODE_FOR_NAME["EXP16_BASE_ANT"] = next_row
    DO._SUB_OPCODE_FOR_NAME["EXP16_POW_ANT"] = next_row + 1
    base_op = _make_op("EXP16_BASE_ANT", base_spec)
    pow_op = _make_op("EXP16_POW_ANT", pow_spec)
    for op in (base_op, pow_op):
        DO.OPS.append(op)
        DO.CUSTOM_DVE_SPECS[op.name] = op.spec
    _exp_ops = (base_op, pow_op)
    return _exp_ops


def build_nc(pairs=PAIRS, seq=S):
    """Build the per-core Bass program (SPMD: same program on all cores)."""
    key = (pairs, seq)
    if key in _nc_cache:
        return _nc_cache[key]

    EXP_BASE, EXP_POW = register_exp_ops()

    NT = seq // P  # seq tiles
    QC = seq // W  # q chunks
    NQT = W // P  # q subtiles per chunk
    NCH = 2  # load chunks per tensor
    CT = NT // NCH  # seq tiles per load chunk
    full = seq == S

    # uniform k-tile groups per q chunk; S^T group tiles rotate through
    # 3 PSUM buffers so QK(g+3) only waits on exp(g)
    groups = [(k0, min(4, NT - k0)) for k0 in range(0, NT, 4)]
    NG = len(groups)
    dve_g = DVE_G if full else set()

    nc = bacc.Bacc("TRN2", target_bir_lowering=False, debug=False)
    Qd = nc.dram_tensor("Q", [pairs, seq, D], F32, kind="ExternalInput").ap()
    Kd = nc.dram_tensor("K", [pairs, seq, D], F32, kind="ExternalInput").ap()
    Vd = nc.dram_tensor("V", [pairs, seq, D], F32, kind="ExternalInput").ap()
    Od = nc.dram_tensor("O", [pairs, seq, D], F32, kind="ExternalOutput").ap()

    with tile.TileContext(nc) as tc:
        with (
            tc.tile_pool(name="consts", bufs=1) as consts,
            tc.tile_pool(name="ld32", bufs=3) as ld32_pool,
            tc.tile_pool(name="ld", bufs=2) as ld_pool,
            tc.tile_pool(name="tr", bufs=3) as tr_pool,
            tc.tile_pool(name="pt", bufs=2) as pt_pool,
            tc.tile_pool(name="tmp", bufs=2) as tmp_pool,
            tc.tile_pool(name="ost", bufs=3) as ost_pool,
            tc.tile_pool(name="sm", bufs=8) as sm_pool,
            tc.tile_pool(name="st_ps", bufs=1, space="PSUM") as st_ps,
            tc.tile_pool(name="o_ps", bufs=2, space="PSUM") as o_ps,
        ):
            # explicit zero bias for exp: a float bias would become a
            # DMA-loaded const AP, entangling every ACTIVATE with a DMA
            # lane semaphore
            zbias = consts.tile([P, 1], F32)
            nc.vector.memset(zbias, 0.0)

            state = {}

            def load_chunk(dst32, src_dram, c, eng=None):
                (eng or nc.sync).dma_start(
                    out=dst32.rearrange("p (t d) -> p t d", d=P)[
                        :, c * CT : (c + 1) * CT
                    ],
                    in_=src_dram.rearrange("(t p) d -> p t d", p=P)[
                        :, c * CT : (c + 1) * CT
                    ],
                )

            def _alloc_32(i, name):
                st = state.setdefault(i, {})
                st[name + "32"] = ld32_pool.tile(
                    [P, seq], F32, tag=name + "32", name=f"{name}32_{i}",
                    bufs=(4 if name == "Vb" else None),
                )
                return st[name + "32"]

            def emit_load(i, name, src_dram, eng=None):
                t = _alloc_32(i, name)
                for c in range(NCH):
                    load_chunk(t, src_dram, c, eng)

            def load_chunk_named(i, name, src_dram, c, eng=None):
                st = state.setdefault(i, {})
                t = _alloc_32(i, name) if c == 0 else st[name + "32"]
                load_chunk(t, src_dram, c, eng)

            def emit_cast(i, name, eng=None, half=None):
                # K/Q casts run on DVE (fp32->fp16 SBUF copies hit the
                # 2x_2p mode there: ~0.7us per half vs 3.6us on GpSimd),
                # chunked in halves so no single op blocks the DVE queue
                # for long.  V stays on GpSimd, which has nothing else.
                st = state[i]
                if half in (None, 0):
                    st[name] = ld_pool.tile(
                        [P, seq], F16, tag=name, name=f"{name}{i}"
                    )
                dst, src = st[name], st[name + "32"]
                eng = eng or nc.gpsimd
                if half is None:
                    eng.tensor_copy(out=dst, in_=src)
                elif eng is nc.scalar:
                    h = seq // 2
                    eng.activation(
                        out=dst[:, half * h : (half + 1) * h],
                        in_=src[:, half * h : (half + 1) * h],
                        func=mybir.ActivationFunctionType.Copy,
                    )
                else:
                    h = seq // 2
                    eng.tensor_copy(
                        out=dst[:, half * h : (half + 1) * h],
                        in_=src[:, half * h : (half + 1) * h],
                    )

            def emit_transpose(i, name, chunk=None, eng=None):
                """Kb/Qb [s, (t d)] f16 -> Kt/Qt [d, (t s)] via DMA XBAR."""
                st = state[i]
                if chunk in (None, 0):
                    st[name + "t"] = tr_pool.tile(
                        [P, seq], F16, tag=name + "t", name=f"{name}t{i}"
                    )
                dst = st[name + "t"]
                src_v = st[name].rearrange("p (t d) -> p t d", d=P)
                dst_v = dst.rearrange("p (t s) -> p t s", s=P)
                if chunk is None:
                    lo, hi = 0, NT
                else:
                    lo, hi = chunk * CT, (chunk + 1) * CT
                (eng or nc.sync).dma_start_transpose(
                    out=dst_v[:, lo:hi],
                    in_=src_v[:, lo:hi].rearrange("p t d -> p (t d)"),
                )

            def emit_cast_V(i, eng=None, half=None):
                eng = eng or nc.gpsimd
                st = state[i]
                if half in (None, 0):
                    st["Vaug"] = ld_pool.tile(
                        [P, NT * DA], F16, tag="Vaug", name=f"Vaug{i}", bufs=3
                    )
                vv = st["Vaug"].rearrange("p (t e) -> p t e", e=DA)
                v32 = st["Vb32"].rearrange("p (t d) -> p t d", d=P)
                if half is None:
                    lo, hi = 0, NT
                else:
                    lo, hi = half * CT, (half + 1) * CT
                eng.tensor_copy(out=vv[:, lo:hi, 0:D], in_=v32[:, lo:hi])
                eng.memset(vv[:, lo:hi, D:DA], 1.0)

            # gap_tasks: global gap index (pair*QC + qc) -> prep closures,
            # emitted right after that q-chunk completes (normalize). Prep
            # that would land before gap 0 is emitted upfront (loads first
            # so the sync queue is not head-of-line blocked by transposes
            # waiting on casts).
            gap_tasks = {}
            upfront_loads = []
            upfront = []

            def schedule(gap, fn, load=False):
                if gap < 0:
                    (upfront_loads if load else upfront).append(fn)
                else:
                    gap_tasks.setdefault(gap, []).append(fn)

            for i in range(pairs):
                base = (i - 1) * QC  # gaps of the previous pair's main loop
                lbase = (i - 2) * QC  # loads / prep go two pairs ahead
                g1 = min(1, max(0, QC - 1))
                g2 = min(2, max(0, QC - 2))
                g4 = min(4, max(0, QC - 1))
                g5 = min(5, max(0, QC - 1))
                g6 = min(6, max(0, QC - 1))
                g3 = min(3, max(0, QC - 1))
                # only pair 0's loads go to the head of the sync queue; later
                # pairs' upfront loads queue behind pair 0's transposes so
                # those aren't head-of-line blocked behind 12 load chunks
                if i == 0:
                    # prologue: loads fan out over four DMA queues (one
                    # hwdge queue moves ~138GB/s; serialized transfers of
                    # 3MB put the first QK at ~26us).  The sync queue gets
                    # only K.c0 so the XBAR transposes behind it start as
                    # soon as the casts land.  Casts spread over the idle
                    # DVE/ACT (fast) with only Vaug.h1 on the slow Pool.
                    schedule(-1, (lambda: emit_load(0, "Kb", Kd[0])), load=True)
                    schedule(-1, (lambda: load_chunk_named(0, "Qb", Qd[0], 0)), load=True)
                    schedule(-1, (lambda: load_chunk_named(0, "Vb", Vd[0], 0)), load=True)
                    schedule(-1, (lambda: load_chunk_named(0, "Qb", Qd[0], 1)), load=True)
                    schedule(-1, (lambda: load_chunk_named(0, "Vb", Vd[0], 1)), load=True)
                    schedule(-1, (lambda: emit_cast(0, "Kb", nc.vector, half=0)))
                    schedule(-1, (lambda: emit_cast(0, "Kb", nc.vector, half=1)))
                    schedule(-1, (lambda: emit_cast(0, "Qb", nc.scalar, half=0)))
                    schedule(-1, (lambda: emit_cast(0, "Qb", nc.vector, half=1)))
                    schedule(-1, (lambda: emit_cast_V(0, nc.vector, half=0)))
                    schedule(-1, (lambda: emit_cast_V(0, nc.gpsimd, half=1)))
                    schedule(-1, (lambda: emit_transpose(0, "Kb", chunk=0)))
                    schedule(-1, (lambda: emit_transpose(0, "Kb", chunk=1)))
                    schedule(-1, (lambda: emit_transpose(0, "Qb", chunk=0)))
                    schedule(-1, (lambda: emit_transpose(0, "Qb", chunk=1)))
                    continue
                if i == 2:
                    # pair 2's loads can't go 2 pairs ahead (there is no
                    # pair before 0); gap-paced loads leave the one DMA
                    # port idle ~11us at the front while pair 2 is
                    # DMA-starved -- stream them upfront instead
                    schedule(-1, (lambda: emit_load(2, "Kb", Kd[2])))
                    schedule(-1, (lambda: emit_load(2, "Qb", Qd[2])))
                    schedule(-1, (lambda: emit_load(2, "Vb", Vd[2])))
                else:
                    schedule(lbase + 0, (lambda i=i: emit_load(i, "Kb", Kd[i])))
                    schedule(lbase + g2, (lambda i=i: emit_load(i, "Qb", Qd[i])))
                    schedule(lbase + g4, (lambda i=i: emit_load(i, "Vb", Vd[i])))
                # casts (GpSimd halves) + DMA transposes (sync) ~1.5 pairs
                # ahead: the transposes sit on the in-order sync queue and
                # wait for the casts, so both must clear well before the
                # pair's first QK matmul.  For i == 1 the gaps clamp into
                # pair 0's early q-chunks (NOT upfront: an upfront cast
                # ahead of pair 0's DVE exp work stalls the PE for ~15us).
                # Casts stay on GpSimd: fast DVE casts couple the DVE
                # queue (exp + normalize) to DMA-load timing and one late
                # load then starves the PE for ~10us.
                cb = max(lbase, 0)
                # prep must be emitted before pair i's first QK: gap c
                # fires at global group-event NG*c + NG-1 + pvq_depth
                last = i * QC - 1 - (4 + NG - 1) // NG

                def sch(gap, fn):
                    schedule(min(gap, last), fn)

                if i == 1:
                    # pair 1's prep compresses into pair 0's window; its
                    # loads complete early (upfront) so fast DVE/ACT casts
                    # are safe here and keep ~14us of work off Pool, which
                    # otherwise overflows and stalls pair 2's transposes
                    sch(cb + 0, (lambda: emit_cast(1, "Kb", nc.vector, half=0)))
                    sch(cb + min(1, QC - 1), (lambda: emit_cast(1, "Kb", nc.vector, half=1)))
                    sch(cb + min(2, QC - 1), (lambda: emit_transpose(1, "Kb", chunk=0)))
                    sch(cb + min(2, QC - 1), (lambda: emit_transpose(1, "Kb", chunk=1)))
                    sch(cb + min(2, QC - 1), (lambda: emit_cast(1, "Qb", nc.scalar, half=0)))
                    sch(cb + min(3, QC - 1), (lambda: emit_cast(1, "Qb", nc.vector, half=1)))
                    sch(cb + min(4, QC - 1), (lambda: emit_transpose(1, "Qb", chunk=0)))
                    sch(cb + min(4, QC - 1), (lambda: emit_transpose(1, "Qb", chunk=1)))
                else:
                    # for pair 2 everything is DMA-bound behind ~9MB of
                    # queued loads; emit its transposes early and let the
                    # sync queue block on the cast sems (nothing behind
                    # it is urgent) rather than gap-pace them ~7us late
                    # pair 2 is DMA-starved: its transposes issue from the
                    # GpSimd queue, naturally right behind its own Pool
                    # casts, instead of being gap-paced ~7us later on sync
                    # pair 2 is DMA/Pool-chain bound: emit its transposes
                    # in the same gap as the cast halves (sync just blocks
                    # on the cast sems; nothing urgent queues behind)
                    te = None
                    kt_g = 2 if i == 2 else 3
                    qt_g = 4 if i == 2 else 5
                    sch(cb + min(1, QC - 1), (lambda i=i: emit_cast(i, "Kb", half=0)))
                    sch(cb + min(2, QC - 1), (lambda i=i: emit_cast(i, "Kb", half=1)))
                    sch(cb + min(kt_g, QC - 1), (lambda i=i, te=te: emit_transpose(i, "Kb", chunk=0, eng=te)))
                    sch(cb + min(kt_g, QC - 1), (lambda i=i, te=te: emit_transpose(i, "Kb", chunk=1, eng=te)))
                    sch(cb + min(3, QC - 1), (lambda i=i: emit_cast(i, "Qb", half=0)))
                    sch(cb + min(4, QC - 1), (lambda i=i: emit_cast(i, "Qb", half=1)))
                    sch(cb + min(qt_g, QC - 1), (lambda i=i, te=te: emit_transpose(i, "Qb", chunk=0, eng=te)))
                    sch(cb + min(qt_g, QC - 1), (lambda i=i, te=te: emit_transpose(i, "Qb", chunk=1, eng=te)))
                if i == 1:
                    # V1's load is upfront (done early): a DVE half is
                    # safe and halves the Pool burst in pair 0's window
                    sch(base + 0, (lambda: emit_cast_V(1, nc.vector, half=0)))
                    sch(base + 0, (lambda: emit_cast_V(1, nc.gpsimd, half=1)))
                elif i == 2:
                    # halved so the first PV of pair 2 only waits ~3.5us
                    sch(base - 2, (lambda: emit_cast_V(2, half=0)))
                    sch(base - 1, (lambda: emit_cast_V(2, half=1)))
                else:
                    sch(base + 0, (lambda i=i: emit_cast_V(i)))

            for fn in upfront_loads:
                fn()
            for fn in upfront:
                fn()

            # ---- global group-stream software pipeline ----
            qc_state = {}

            def finish_qc(i, qc):
                """Normalize + prep tasks + (if last qc) store for one q-chunk."""
                stq = qc_state.pop((i, qc))
                o_t = stq["o"]
                o_view = o_t[:, 0 : NQT * DA].rearrange("p (q e) -> p q e", e=DA)
                Ost = state[i]["Ost"]
                rec = sm_pool.tile([P, NQT], F32, tag="rec", name=f"rec{i}_{qc}")
                nc.vector.reciprocal(out=rec, in_=o_view[:, :, D : D + 1])
                for qt in range(NQT):
                    t = qc * NQT + qt
                    nc.vector.tensor_scalar_mul(
                        Ost[:, t * P : (t + 1) * P],
                        o_view[:, qt, 0:D],
                        rec[:, qt : qt + 1],
                    )
                # store finished q-tiles in chunks so the last pair's store
                # doesn't serialize behind all 8 normalizes (epilogue tail);
                # the last pair stores every q-chunk to shorten the tail
                if qc % 2 == 1 or qc == QC - 1 or i == pairs - 1:
                    t0 = state[i].get("stored_t", 0)
                    t1 = (qc + 1) * NQT
                    state[i]["stored_t"] = t1
                    nc.sync.dma_start(
                        out=Od[i].rearrange("(t p) d -> p t d", p=P)[:, t0:t1],
                        in_=Ost.rearrange("p (t d) -> p t d", d=P)[:, t0:t1],
                    )
                for fn in gap_tasks.pop(i * QC + qc, []):
                    fn()

            def emit_pv(ev, pt_tile):
                i, qc, k0, gk = ev
                o_t = qc_state[(i, qc)]["o"]
                Vaug = state[i]["Vaug"]
                for j in range(gk):
                    kt = k0 + j
                    for qt in range(NQT):
                        nc.tensor.matmul(
                            o_t[:, qt * DA : (qt + 1) * DA],
                            lhsT=pt_tile[:, j * W + qt * P : j * W + (qt + 1) * P],
                            rhs=Vaug[:, kt * DA : (kt + 1) * DA],
                            start=(kt == 0 and qt == 0),
                            stop=(kt == NT - 1 and qt == NQT - 1),
                        )
                if k0 + gk == NT:
                    finish_qc(i, qc)

            events = [
                (i, qc, g)
                for i in range(pairs)
                for qc in range(QC)
                for g in range(NG)
            ]
            pvq = []
            for i, qc, g in events:
                k0, gk = groups[g]
                if g == 0:
                    if qc == 0:
                        state[i]["Ost"] = ost_pool.tile(
                            [P, seq], F32, tag="Ost", name=f"Ost{i}"
                        )
                    qc_state[(i, qc)] = {
                        "o": o_ps.tile([P, 512], F32, tag="o", name=f"o{i}_{qc}")
                    }
                Qt, Kt = state[i]["Qbt"], state[i]["Kbt"]
                stp = st_ps.tile(
                    [P, gk * W], F32, tag="st", name=f"st{i}_{qc}_{k0}", bufs=3
                )
                for j in range(gk):
                    kt = k0 + j
                    nc.tensor.matmul(
                        stp[:, j * W : (j + 1) * W],
                        lhsT=Kt[:, kt * P : (kt + 1) * P],
                        rhs=Qt[:, qc * W : (qc + 1) * W],
                        start=True,
                        stop=True,
                    )
                # the last pair's last DVE chunk would put a 2.4us DVE
                # chain on the epilogue critical path; use ACT there
                use_dve = (
                    (qc, g) in dve_g and gk * W == 1024
                    and not (i == pairs - 1 and qc == QC - 1)
                )
                pt = pt_pool.tile(
                    [P, gk * W], F16, tag="pt", name=f"pt{i}_{qc}_{k0}", bufs=5
                )
                # q-chunks with no DVE group overload ACT (4 groups =
                # 4.45us > the PE's ~4.2us per chunk): split their last
                # group's exp between ACT and DVE
                use_split = (
                    full and not use_dve and g == NG - 1 and gk * W == 1024
                    and not (i == pairs - 1 and qc == QC - 1)
                )
                if use_dve:
                    tmp = tmp_pool.tile(
                        [P, gk * W], F16, tag="tmp", name=f"tmp{i}_{qc}_{k0}"
                    )
                    nc.vector._custom_dve(
                        EXP_BASE, out=tmp, in0=stp,
                        s0=SCALE / 16.0, s1=EXP_C2, imm2=EXP_C3,
                    )
                    nc.vector._custom_dve(EXP_POW, out=pt, in0=tmp)
                elif use_split:
                    h = (gk * W) // 2
                    nc.scalar.activation(
                        out=pt[:, 0:h],
                        in_=stp[:, 0:h],
                        func=mybir.ActivationFunctionType.Exp,
                        bias=zbias[:, 0:1],
                        scale=SCALE,
                    )
                    tmp = tmp_pool.tile(
                        [P, h], F16, tag="tmph", name=f"tmph{i}_{qc}_{k0}"
                    )
                    nc.vector._custom_dve(
                        EXP_BASE, out=tmp, in0=stp[:, h:],
                        s0=SCALE / 16.0, s1=EXP_C2, imm2=EXP_C3,
                    )
                    nc.vector._custom_dve(EXP_POW, out=pt[:, h:], in0=tmp)
                else:
                    nc.scalar.activation(
                        out=pt,
                        in_=stp,
                        func=mybir.ActivationFunctionType.Exp,
                        bias=zbias[:, 0:1],
                        scale=SCALE,
                    )
                pvq.append(((i, qc, k0, gk), pt))
                if len(pvq) > 3:
                    emit_pv(*pvq.pop(0))
            while pvq:
                emit_pv(*pvq.pop(0))

    nc.compile()
    _nc_cache[key] = nc
    return nc


def run(Q, K, V, trace=False):
    """Run on 8 cores; Q/K/V are full [B,H,S,D] fp32 arrays.

    Returns (output [B,H,S,D] fp32, BassKernelResults)."""
    Qf = np.ascontiguousarray(np.asarray(Q, dtype=np.float32).reshape(B * H, S, D))
    Kf = np.ascontiguousarray(np.asarray(K, dtype=np.float32).reshape(B * H, S, D))
    Vf = np.ascontiguousarray(np.asarray(V, dtype=np.float32).reshape(B * H, S, D))

    nc = build_nc()
    in_maps = [
        {
            "Q": Qf[c * PAIRS : (c + 1) * PAIRS],
            "K": Kf[c * PAIRS : (c + 1) * PAIRS],
            "V": Vf[c * PAIRS : (c + 1) * PAIRS],
        }
        for c in range(N_CORES)
    ]
    res = run_bass_kernel_spmd(nc, in_maps, list(range(N_CORES)), trace=trace)
    out = np.concatenate([res.results[c]["O"] for c in range(N_CORES)], axis=0)
    return out.reshape(B, H, S, D), res


def kernel(Q, K, V):
    # never trace in the grading path (the NTFF hook isn't available
    # outside our own test harness)
    prev = os.environ.get("BASS_NEVER_TRACE")
    os.environ["BASS_NEVER_TRACE"] = "1"
    try:
        out, _ = run(Q, K, V, trace=False)
    finally:
        if prev is None:
            os.environ.pop("BASS_NEVER_TRACE", None)
        else:
            os.environ["BASS_NEVER_TRACE"] = prev
    return out


# revision 35
# speedup vs baseline: 1.0206x; 1.0206x over previous
"""Multi-head attention kernel for Trainium2, sharded over 8 NeuronCores.

Problem: Q,K,V [4, 16, 2048, 128] fp32 -> softmax(Q K^T / sqrt(128)) V.

Sharding: the 4*16 = 64 (batch, head) pairs are split across 8 cores,
8 pairs per core (pure data parallelism, no collectives).

Per-core kernel (flash-attention style, S^T layout), v3:
  - Q, K are loaded fp32, cast to fp16 (Q on DVE in 2 chunks, K on
    GpSimd), and transposed into Qt/Kt [d=128, seq] layout by the DMA
    XBAR (dma_start_transpose) -- no PE or PSUM involvement.
  - V is cast into V_aug [k, 129] tiles on GpSimd whose last column is
    1.0 (so the PV matmul also produces the softmax row sums for free).
  - Per q-chunk (W=256) the 16 k-tiles form 4 uniform groups of 4;
    S^T[k, q] group tiles [128, 1024] land in PSUM, rotating through 3
    two-bank buffers so QK(g+3) only waits on exp(g) -- deeper
    decoupling than the old (4,8,4)/A,B split and shorter exp latency
    per group, which removes the ~1.2us PE stalls at group boundaries.
  - exp(S^T/sqrt(d)): most groups go to the ACT engine (fp32 PSUM ->
    fp16 P^T in SBUF); DVE_G groups instead run on the Vector engine
    via two custom DVE ops: base = 1 + y(1 + y(c2 + y c3)) with
    y = s*scale/16, then base^16 (rel err ~2e-4 rms).  This keeps the
    saturated ACT engine just under the PE's per-pair time.
  - O_unnorm[q, 0:128] and row sums [q, 128] accumulate in PSUM over
    all k tiles via matmul(lhsT=P^T slice, rhs=V_aug).
  - Final normalize: reciprocal of sums on DVE, the O*recip scaling on
    GpSimd (keeps the DVE queue free for exp).
  - PSUM budget (8 banks): st 3x2 + two O accumulators 2 = 8.

Scheduling: a global software pipeline over the group stream (PV
matmuls trail the S^T/exp stream by 3 groups), with prep for later
pairs (loads two pairs ahead; casts + DMA transposes ~1.5 pairs ahead)
emitted into the per-q-chunk gaps so the ACT/PE pipeline never drains
at pair boundaries.
"""

import os
import re
import sys

for _p in ("/opt/trn_rl_repo",):
    if _p not in sys.path and os.path.isdir(_p):
        sys.path.insert(0, _p)

import numpy as np

import concourse.bass as bass
import concourse.bacc as bacc
import concourse.tile as tile
from concourse import mybir
from concourse import dve_ops as DO
from concourse.dve_spec import Spec, Src0, C0, C1, C2, One, sq
from concourse.dve_table_gen import dve_ver_for
from concourse.bass_utils import run_bass_kernel_spmd

F32 = mybir.dt.float32
F16 = mybir.dt.float16

B, H, S, D = 4, 16, 2048, 128
N_CORES = 8
PAIRS = (B * H) // N_CORES  # (b,h) pairs per core
P = 128  # partition dim / head dim / seq tile
W = 256  # q-chunk width (one PSUM bank of O accum incl. sums col)
DA = D + 1  # V augmented with a ones column
SCALE = float(1.0 / np.sqrt(D))
# deg-3 base coefficients for the DVE exp (Taylor; composite ^16)
EXP_C2, EXP_C3 = 0.5, 1.0 / 6.0
# (qc, group) pairs whose exp runs on DVE instead of ACT.  Group 3 so
# the previous q-chunk's normalize (same DVE queue) is emitted before
# the exp chain -- the normalize gates the O PSUM bank rotation and
# must not sit behind a 2.4us exp chain.
DVE_G = {(1, 3), (2, 3), (3, 3), (5, 3), (6, 3), (7, 3)}

_nc_cache = {}
_exp_ops = None


def _make_op(name, spec):
    """Build a DveOp with the correct uops_sha (sha comes from the
    drift error message on first compile)."""
    ver = dve_ver_for("TRN2")
    op = DO.DveOp(name, spec, subdim=False, uops_sha={ver: "?"})
    try:
        op.compile(ver)
    except ValueError as e:
        sha = re.search(r"([0-9a-f]{16,})", str(e)).group(1)
        op = DO.DveOp(name, spec, subdim=False, uops_sha={ver: sha})
    op.compile(ver)
    return op


def register_exp_ops():
    """Register the two custom DVE ops for exp(x*scale) = base(y)^16.

    base: 1 + y*(1 + y*(C1 + y*C2)), y = Src0*C0  (7 ALU stages)
    pow:  Src0^16                                  (4 ALU stages)
    """
    global _exp_ops
    if _exp_ops is not None:
        return _exp_ops
    if "EXP16_BASE_ANT" in DO._SUB_OPCODE_FOR_NAME:
        ops = {o.name: o for o in DO.OPS}
        _exp_ops = (ops["EXP16_BASE_ANT"], ops["EXP16_POW_ANT"])
        return _exp_ops
    y = Src0 * C0
    t = ((y * C2) + C1) * y
    p = ((t + One) * y) + One
    base_spec = Spec(
        body=p,
        reference=lambda in0, in1, s0, s1, imm2: (
            lambda yy: 1.0 + yy * (1.0 + yy * (s1 + yy * imm2))
        )(np.asarray(in0, np.float32) * (s0 if np.isscalar(s0) else s0)),
    )
    pow_spec = Spec(
        body=sq(sq(sq(sq(Src0)))),
        reference=lambda in0, in1, s0, s1, imm2: np.asarray(in0, np.float32) ** 16,
    )
    next_row = DO._CUSTOM_DVE_ROW_BASE + len(DO.OPS)
    DO._SUB_OPCODE_FOR_NAME["EXP16_BASE_ANT"] = next_row
    DO._SUB_OPCODE_FOR_NAME["EXP16_POW_ANT"] = next_row + 1
    base_op = _make_op("EXP16_BASE_ANT", base_spec)
    pow_op = _make_op("EXP16_POW_ANT", pow_spec)
    for op in (base_op, pow_op):
        DO.OPS.append(op)
        DO.CUSTOM_DVE_SPECS[op.name] = op.spec
    _exp_ops = (base_op, pow_op)
    return _exp_ops


def build_nc(pairs=PAIRS, seq=S):
    """Build the per-core Bass program (SPMD: same program on all cores)."""
    key = (pairs, seq)
    if key in _nc_cache:
        return _nc_cache[key]

    EXP_BASE, EXP_POW = register_exp_ops()

    NT = seq // P  # seq tiles
    QC = seq // W  # q chunks
    NQT = W // P  # q subtiles per chunk
    NCH = 2  # load chunks per tensor
    CT = NT // NCH  # seq tiles per load chunk
    full = seq == S

    # uniform k-tile groups per q chunk; S^T group tiles rotate through
    # 3 PSUM buffers so QK(g+3) only waits on exp(g)
    groups = [(k0, min(4, NT - k0)) for k0 in range(0, NT, 4)]
    NG = len(groups)
    dve_g = DVE_G if full else set()

    nc = bacc.Bacc("TRN2", target_bir_lowering=False, debug=False)
    Qd = nc.dram_tensor("Q", [pairs, seq, D], F32, kind="ExternalInput").ap()
    Kd = nc.dram_tensor("K", [pairs, seq, D], F32, kind="ExternalInput").ap()
    Vd = nc.dram_tensor("V", [pairs, seq, D], F32, kind="ExternalInput").ap()
    Od = nc.dram_tensor("O", [pairs, seq, D], F32, kind="ExternalOutput").ap()

    with tile.TileContext(nc) as tc:
        with (
            tc.tile_pool(name="consts", bufs=1) as consts,
            tc.tile_pool(name="ld32", bufs=3) as ld32_pool,
            tc.tile_pool(name="ld", bufs=2) as ld_pool,
            tc.tile_pool(name="tr", bufs=3) as tr_pool,
            tc.tile_pool(name="pt", bufs=2) as pt_pool,
            tc.tile_pool(name="tmp", bufs=2) as tmp_pool,
            tc.tile_pool(name="ost", bufs=3) as ost_pool,
            tc.tile_pool(name="sm", bufs=8) as sm_pool,
            tc.tile_pool(name="st_ps", bufs=1, space="PSUM") as st_ps,
            tc.tile_pool(name="o_ps", bufs=2, space="PSUM") as o_ps,
        ):
            # explicit zero bias for exp: a float bias would become a
            # DMA-loaded const AP, entangling every ACTIVATE with a DMA
            # lane semaphore
            zbias = consts.tile([P, 1], F32)
            nc.vector.memset(zbias, 0.0)

            state = {}

            def load_chunk(dst32, src_dram, c, eng=None):
                (eng or nc.sync).dma_start(
                    out=dst32.rearrange("p (t d) -> p t d", d=P)[
                        :, c * CT : (c + 1) * CT
                    ],
                    in_=src_dram.rearrange("(t p) d -> p t d", p=P)[
                        :, c * CT : (c + 1) * CT
                    ],
                )

            def _alloc_32(i, name):
                st = state.setdefault(i, {})
                st[name + "32"] = ld32_pool.tile(
                    [P, seq], F32, tag=name + "32", name=f"{name}32_{i}",
                    bufs=(4 if name == "Vb" else None),
                )
                return st[name + "32"]

            def emit_load(i, name, src_dram, eng=None):
                t = _alloc_32(i, name)
                for c in range(NCH):
                    load_chunk(t, src_dram, c, eng)

            def load_chunk_named(i, name, src_dram, c, eng=None):
                st = state.setdefault(i, {})
                t = _alloc_32(i, name) if c == 0 else st[name + "32"]
                load_chunk(t, src_dram, c, eng)

            def emit_cast(i, name, eng=None, half=None):
                # K/Q casts run on DVE (fp32->fp16 SBUF copies hit the
                # 2x_2p mode there: ~0.7us per half vs 3.6us on GpSimd),
                # chunked in halves so no single op blocks the DVE queue
                # for long.  V stays on GpSimd, which has nothing else.
                st = state[i]
                if half in (None, 0):
                    st[name] = ld_pool.tile(
                        [P, seq], F16, tag=name, name=f"{name}{i}"
                    )
                dst, src = st[name], st[name + "32"]
                eng = eng or nc.gpsimd
                if half is None:
                    eng.tensor_copy(out=dst, in_=src)
                elif eng is nc.scalar:
                    h = seq // 2
                    eng.activation(
                        out=dst[:, half * h : (half + 1) * h],
                        in_=src[:, half * h : (half + 1) * h],
                        func=mybir.ActivationFunctionType.Copy,
                    )
                else:
                    h = seq // 2
                    eng.tensor_copy(
                        out=dst[:, half * h : (half + 1) * h],
                        in_=src[:, half * h : (half + 1) * h],
                    )

            def emit_transpose(i, name, chunk=None, eng=None):
                """Kb/Qb [s, (t d)] f16 -> Kt/Qt [d, (t s)] via DMA XBAR."""
                st = state[i]
                if chunk in (None, 0):
                    st[name + "t"] = tr_pool.tile(
                        [P, seq], F16, tag=name + "t", name=f"{name}t{i}"
                    )
                dst = st[name + "t"]
                src_v = st[name].rearrange("p (t d) -> p t d", d=P)
                dst_v = dst.rearrange("p (t s) -> p t s", s=P)
                if chunk is None:
                    lo, hi = 0, NT
                else:
                    lo, hi = chunk * CT, (chunk + 1) * CT
                (eng or nc.sync).dma_start_transpose(
                    out=dst_v[:, lo:hi],
                    in_=src_v[:, lo:hi].rearrange("p t d -> p (t d)"),
                )

            def emit_cast_V(i, eng=None, half=None):
                eng = eng or nc.gpsimd
                st = state[i]
                if half in (None, 0):
                    st["Vaug"] = ld_pool.tile(
                        [P, NT * DA], F16, tag="Vaug", name=f"Vaug{i}", bufs=3
                    )
                vv = st["Vaug"].rearrange("p (t e) -> p t e", e=DA)
                v32 = st["Vb32"].rearrange("p (t d) -> p t d", d=P)
                if half is None:
                    lo, hi = 0, NT
                else:
                    lo, hi = half * CT, (half + 1) * CT
                eng.tensor_copy(out=vv[:, lo:hi, 0:D], in_=v32[:, lo:hi])
                eng.memset(vv[:, lo:hi, D:DA], 1.0)

            # gap_tasks: global gap index (pair*QC + qc) -> prep closures,
            # emitted right after that q-chunk completes (normalize). Prep
            # that would land before gap 0 is emitted upfront (loads first
            # so the sync queue is not head-of-line blocked by transposes
            # waiting on casts).
            gap_tasks = {}
            upfront_loads = []
            upfront = []

            def schedule(gap, fn, load=False):
                if gap < 0:
                    (upfront_loads if load else upfront).append(fn)
                else:
                    gap_tasks.setdefault(gap, []).append(fn)

            for i in range(pairs):
                base = (i - 1) * QC  # gaps of the previous pair's main loop
                lbase = (i - 2) * QC  # loads / prep go two pairs ahead
                g1 = min(1, max(0, QC - 1))
                g2 = min(2, max(0, QC - 2))
                g4 = min(4, max(0, QC - 1))
                g5 = min(5, max(0, QC - 1))
                g6 = min(6, max(0, QC - 1))
                g3 = min(3, max(0, QC - 1))
                # only pair 0's loads go to the head of the sync queue; later
                # pairs' upfront loads queue behind pair 0's transposes so
                # those aren't head-of-line blocked behind 12 load chunks
                if i == 0:
                    # prologue: loads fan out over four DMA queues (one
                    # hwdge queue moves ~138GB/s; serialized transfers of
                    # 3MB put the first QK at ~26us).  The sync queue gets
                    # only K.c0 so the XBAR transposes behind it start as
                    # soon as the casts land.  Casts spread over the idle
                    # DVE/ACT (fast) with only Vaug.h1 on the slow Pool.
                    schedule(-1, (lambda: emit_load(0, "Kb", Kd[0])), load=True)
                    schedule(-1, (lambda: load_chunk_named(0, "Qb", Qd[0], 0)), load=True)
                    schedule(-1, (lambda: load_chunk_named(0, "Vb", Vd[0], 0)), load=True)
                    schedule(-1, (lambda: load_chunk_named(0, "Qb", Qd[0], 1)), load=True)
                    schedule(-1, (lambda: load_chunk_named(0, "Vb", Vd[0], 1)), load=True)
                    schedule(-1, (lambda: emit_cast(0, "Kb", nc.vector, half=0)))
                    schedule(-1, (lambda: emit_cast(0, "Kb", nc.vector, half=1)))
                    schedule(-1, (lambda: emit_cast(0, "Qb", nc.scalar, half=0)))
                    schedule(-1, (lambda: emit_cast(0, "Qb", nc.vector, half=1)))
                    schedule(-1, (lambda: emit_cast_V(0, nc.vector, half=0)))
                    schedule(-1, (lambda: emit_cast_V(0, nc.gpsimd, half=1)))
                    schedule(-1, (lambda: emit_transpose(0, "Kb", chunk=0)))
                    schedule(-1, (lambda: emit_transpose(0, "Kb", chunk=1)))
                    schedule(-1, (lambda: emit_transpose(0, "Qb", chunk=0)))
                    schedule(-1, (lambda: emit_transpose(0, "Qb", chunk=1)))
                    continue
                if i == 2:
                    # pair 2's loads can't go 2 pairs ahead (there is no
                    # pair before 0); gap-paced loads leave the one DMA
                    # port idle ~11us at the front while pair 2 is
                    # DMA-starved -- stream them upfront instead
                    schedule(-1, (lambda: emit_load(2, "Kb", Kd[2])))
                    schedule(-1, (lambda: emit_load(2, "Qb", Qd[2])))
                    schedule(-1, (lambda: emit_load(2, "Vb", Vd[2])))
                else:
                    schedule(lbase + 0, (lambda i=i: emit_load(i, "Kb", Kd[i])))
                    schedule(lbase + g2, (lambda i=i: emit_load(i, "Qb", Qd[i])))
                    schedule(lbase + g4, (lambda i=i: emit_load(i, "Vb", Vd[i])))
                # casts (GpSimd halves) + DMA transposes (sync) ~1.5 pairs
                # ahead: the transposes sit on the in-order sync queue and
                # wait for the casts, so both must clear well before the
                # pair's first QK matmul.  For i == 1 the gaps clamp into
                # pair 0's early q-chunks (NOT upfront: an upfront cast
                # ahead of pair 0's DVE exp work stalls the PE for ~15us).
                # Casts stay on GpSimd: fast DVE casts couple the DVE
                # queue (exp + normalize) to DMA-load timing and one late
                # load then starves the PE for ~10us.
                cb = max(lbase, 0)
                # prep must be emitted before pair i's first QK: gap c
                # fires at global group-event NG*c + NG-1 + pvq_depth
                last = i * QC - 1 - (4 + NG - 1) // NG

                def sch(gap, fn):
                    schedule(min(gap, last), fn)

                if i == 1:
                    # pair 1's prep compresses into pair 0's window; its
                    # loads complete early (upfront) so fast DVE/ACT casts
                    # are safe here and keep ~14us of work off Pool, which
                    # otherwise overflows and stalls pair 2's transposes
                    sch(cb + 0, (lambda: emit_cast(1, "Kb", nc.vector, half=0)))
                    sch(cb + min(1, QC - 1), (lambda: emit_cast(1, "Kb", nc.vector, half=1)))
                    sch(cb + min(2, QC - 1), (lambda: emit_transpose(1, "Kb", chunk=0)))
                    sch(cb + min(2, QC - 1), (lambda: emit_transpose(1, "Kb", chunk=1)))
                    sch(cb + min(2, QC - 1), (lambda: emit_cast(1, "Qb", nc.scalar, half=0)))
                    sch(cb + min(3, QC - 1), (lambda: emit_cast(1, "Qb", nc.vector, half=1)))
                    sch(cb + min(4, QC - 1), (lambda: emit_transpose(1, "Qb", chunk=0)))
                    sch(cb + min(4, QC - 1), (lambda: emit_transpose(1, "Qb", chunk=1)))
                else:
                    # for pair 2 everything is DMA-bound behind ~9MB of
                    # queued loads; emit its transposes early and let the
                    # sync queue block on the cast sems (nothing behind
                    # it is urgent) rather than gap-pace them ~7us late
                    # pair 2 is DMA-starved: its transposes issue from the
                    # GpSimd queue, naturally right behind its own Pool
                    # casts, instead of being gap-paced ~7us later on sync
                    te = None
                    kt_g = 3
                    qt_g = 5
                    sch(cb + min(1, QC - 1), (lambda i=i: emit_cast(i, "Kb", half=0)))
                    sch(cb + min(2, QC - 1), (lambda i=i: emit_cast(i, "Kb", half=1)))
                    sch(cb + min(kt_g, QC - 1), (lambda i=i, te=te: emit_transpose(i, "Kb", chunk=0, eng=te)))
                    sch(cb + min(kt_g, QC - 1), (lambda i=i, te=te: emit_transpose(i, "Kb", chunk=1, eng=te)))
                    sch(cb + min(3, QC - 1), (lambda i=i: emit_cast(i, "Qb", half=0)))
                    sch(cb + min(4, QC - 1), (lambda i=i: emit_cast(i, "Qb", half=1)))
                    sch(cb + min(qt_g, QC - 1), (lambda i=i, te=te: emit_transpose(i, "Qb", chunk=0, eng=te)))
                    sch(cb + min(qt_g, QC - 1), (lambda i=i, te=te: emit_transpose(i, "Qb", chunk=1, eng=te)))
                if i == 1:
                    # V1's load is upfront (done early): a DVE half is
                    # safe and halves the Pool burst in pair 0's window
                    sch(base + 0, (lambda: emit_cast_V(1, nc.vector, half=0)))
                    sch(base + 0, (lambda: emit_cast_V(1, nc.gpsimd, half=1)))
                elif i == 2:
                    # halved so the first PV of pair 2 only waits ~3.5us
                    sch(base - 2, (lambda: emit_cast_V(2, half=0)))
                    sch(base - 1, (lambda: emit_cast_V(2, half=1)))
                else:
                    sch(base + 0, (lambda i=i: emit_cast_V(i)))

            for fn in upfront_loads:
                fn()
            for fn in upfront:
                fn()

            # ---- global group-stream software pipeline ----
            qc_state = {}

            def finish_qc(i, qc):
                """Normalize + prep tasks + (if last qc) store for one q-chunk."""
                stq = qc_state.pop((i, qc))
                o_t = stq["o"]
                o_view = o_t[:, 0 : NQT * DA].rearrange("p (q e) -> p q e", e=DA)
                Ost = state[i]["Ost"]
                rec = sm_pool.tile([P, NQT], F32, tag="rec", name=f"rec{i}_{qc}")
                nc.vector.reciprocal(out=rec, in_=o_view[:, :, D : D + 1])
                for qt in range(NQT):
                    t = qc * NQT + qt
                    nc.vector.tensor_scalar_mul(
                        Ost[:, t * P : (t + 1) * P],
                        o_view[:, qt, 0:D],
                        rec[:, qt : qt + 1],
                    )
                # store finished q-tiles in chunks so the last pair's store
                # doesn't serialize behind all 8 normalizes (epilogue tail);
                # the last pair stores every q-chunk to shorten the tail
                if qc % 2 == 1 or qc == QC - 1 or i == pairs - 1:
                    t0 = state[i].get("stored_t", 0)
                    t1 = (qc + 1) * NQT
                    state[i]["stored_t"] = t1
                    nc.sync.dma_start(
                        out=Od[i].rearrange("(t p) d -> p t d", p=P)[:, t0:t1],
                        in_=Ost.rearrange("p (t d) -> p t d", d=P)[:, t0:t1],
                    )
                for fn in gap_tasks.pop(i * QC + qc, []):
                    fn()

            def emit_pv(ev, pt_tile):
                i, qc, k0, gk = ev
                o_t = qc_state[(i, qc)]["o"]
                Vaug = state[i]["Vaug"]
                for j in range(gk):
                    kt = k0 + j
                    for qt in range(NQT):
                        nc.tensor.matmul(
                            o_t[:, qt * DA : (qt + 1) * DA],
                            lhsT=pt_tile[:, j * W + qt * P : j * W + (qt + 1) * P],
                            rhs=Vaug[:, kt * DA : (kt + 1) * DA],
                            start=(kt == 0 and qt == 0),
                            stop=(kt == NT - 1 and qt == NQT - 1),
                        )
                if k0 + gk == NT:
                    finish_qc(i, qc)

            events = [
                (i, qc, g)
                for i in range(pairs)
                for qc in range(QC)
                for g in range(NG)
            ]
            pvq = []
            for i, qc, g in events:
                k0, gk = groups[g]
                if g == 0:
                    if qc == 0:
                        state[i]["Ost"] = ost_pool.tile(
                            [P, seq], F32, tag="Ost", name=f"Ost{i}"
                        )
                    qc_state[(i, qc)] = {
                        "o": o_ps.tile([P, 512], F32, tag="o", name=f"o{i}_{qc}")
                    }
                Qt, Kt = state[i]["Qbt"], state[i]["Kbt"]
                stp = st_ps.tile(
                    [P, gk * W], F32, tag="st", name=f"st{i}_{qc}_{k0}", bufs=3
                )
                for j in range(gk):
                    kt = k0 + j
                    nc.tensor.matmul(
                        stp[:, j * W : (j + 1) * W],
                        lhsT=Kt[:, kt * P : (kt + 1) * P],
                        rhs=Qt[:, qc * W : (qc + 1) * W],
                        start=True,
                        stop=True,
                    )
                # the last pair's last DVE chunk would put a 2.4us DVE
                # chain on the epilogue critical path; use ACT there
                use_dve = (
                    (qc, g) in dve_g and gk * W == 1024
                    and not (i == pairs - 1 and qc == QC - 1)
                )
                pt = pt_pool.tile(
                    [P, gk * W], F16, tag="pt", name=f"pt{i}_{qc}_{k0}", bufs=5
                )
                # q-chunks with no DVE group overload ACT (4 groups =
                # 4.45us > the PE's ~4.2us per chunk): split their last
                # group's exp between ACT and DVE
                use_split = (
                    full and not use_dve and g == NG - 1 and gk * W == 1024
                    and not (i == pairs - 1 and qc == QC - 1)
                )
                if use_dve:
                    tmp = tmp_pool.tile(
                        [P, gk * W], F16, tag="tmp", name=f"tmp{i}_{qc}_{k0}"
                    )
                    nc.vector._custom_dve(
                        EXP_BASE, out=tmp, in0=stp,
                        s0=SCALE / 16.0, s1=EXP_C2, imm2=EXP_C3,
                    )
                    nc.vector._custom_dve(EXP_POW, out=pt, in0=tmp)
                elif use_split:
                    h = (gk * W) // 2
                    nc.scalar.activation(
                        out=pt[:, 0:h],
                        in_=stp[:, 0:h],
                        func=mybir.ActivationFunctionType.Exp,
                        bias=zbias[:, 0:1],
                        scale=SCALE,
                    )
                    tmp = tmp_pool.tile(
                        [P, h], F16, tag="tmph", name=f"tmph{i}_{qc}_{k0}"
                    )
                    nc.vector._custom_dve(
                        EXP_BASE, out=tmp, in0=stp[:, h:],
                        s0=SCALE / 16.0, s1=EXP_C2, imm2=EXP_C3,
                    )
                    nc.vector._custom_dve(EXP_POW, out=pt[:, h:], in0=tmp)
                else:
                    nc.scalar.activation(
                        out=pt,
                        in_=stp,
                        func=mybir.ActivationFunctionType.Exp,
                        bias=zbias[:, 0:1],
                        scale=SCALE,
                    )
                pvq.append(((i, qc, k0, gk), pt))
                if len(pvq) > 3:
                    emit_pv(*pvq.pop(0))
            while pvq:
                emit_pv(*pvq.pop(0))

    nc.compile()
    _nc_cache[key] = nc
    return nc


def run(Q, K, V, trace=False):
    """Run on 8 cores; Q/K/V are full [B,H,S,D] fp32 arrays.

    Returns (output [B,H,S,D] fp32, BassKernelResults)."""
    Qf = np.ascontiguousarray(np.asarray(Q, dtype=np.float32).reshape(B * H, S, D))
    Kf = np.ascontiguousarray(np.asarray(K, dtype=np.float32).reshape(B * H, S, D))
    Vf = np.ascontiguousarray(np.asarray(V, dtype=np.float32).reshape(B * H, S, D))

    nc = build_nc()
    in_maps = [
        {
            "Q": Qf[c * PAIRS : (c + 1) * PAIRS],
            "K": Kf[c * PAIRS : (c + 1) * PAIRS],
            "V": Vf[c * PAIRS : (c + 1) * PAIRS],
        }
        for c in range(N_CORES)
    ]
    res = run_bass_kernel_spmd(nc, in_maps, list(range(N_CORES)), trace=trace)
    out = np.concatenate([res.results[c]["O"] for c in range(N_CORES)], axis=0)
    return out.reshape(B, H, S, D), res


def kernel(Q, K, V):
    # never trace in the grading path (the NTFF hook isn't available
    # outside our own test harness)
    prev = os.environ.get("BASS_NEVER_TRACE")
    os.environ["BASS_NEVER_TRACE"] = "1"
    try:
        out, _ = run(Q, K, V, trace=False)
    finally:
        if prev is None:
            os.environ.pop("BASS_NEVER_TRACE", None)
        else:
            os.environ["BASS_NEVER_TRACE"] = prev
    return out
